# revision 34
# baseline (speedup 1.0000x reference)
"""Trainium2 Bass kernel for nn_AutoSlicingModel (segment_reduce).

Computation (per batch item):
  stmt_emb[s]  = mean of hidden_states over the 8 contiguous tokens of statement s
  var_emb      = mean of hidden_states rows at variables_ids (8 occurrences)
  paired[s]    = [stmt_emb[s], var_emb]           (2H = 2048)
  back_preds   = SliceMLP_back(paired[0:128])     (3-layer MLP, gelu/gelu/sigmoid)
  fwd_preds    = SliceMLP_fwd (paired[129:256])
  out          = concat([back_preds, fwd_preds])  -> [B, 255]

Distribution: data-parallel over batch B=64 across 8 NeuronCores (8 items/core),
MLP weights replicated, no cross-core communication; host concatenates.

Device strategy (all-bf16 with fp32 PSUM accumulation; measured max-abs error
vs the fp32 reference is ~1e-4 because the logits are tiny):
  * Pooling on the tensor engine in "transposed" orientation:
      stmtT[h, 16 segs] = X_chunk[128tok, 128h].T @ PT[128tok, 16seg]
    so downstream MLP matmuls need no transposes at all.
  * The variable-token mean is a second tiny matmul per chunk against a
    host-built count-matrix column (handles duplicate ids).
  * var_emb enters layer 1 only through (var @ W1b + b1), which is constant
    per item, so it is computed once per item as a per-partition activation
    bias (bias1T) -- this halves layer-1 FLOPs.
  * Items are processed in quads so every MLP matmul has a 512-wide moving
    operand (full PSUM bank, best PE efficiency).
"""

import sys

if "/opt/trn_rl_repo" not in sys.path:
    sys.path.insert(0, "/opt/trn_rl_repo")

from contextlib import ExitStack

import ml_dtypes
import numpy as np

import concourse.bacc as bacc
import concourse.bass as bass
import concourse.mybir as mybir
import concourse.tile as tile
from concourse.bass_utils import run_bass_kernel_spmd

BF16 = mybir.dt.bfloat16
F32 = mybir.dt.float32
NPBF16 = np.dtype(ml_dtypes.bfloat16)

B, T, H, S, V = 64, 2048, 1024, 256, 8
VAR_LINE = 128
NCORES = 8
IPC = B // NCORES        # items per core = 8
NQUAD = IPC // 4         # quads per core = 2
NT = T // 128            # 16 token tiles per item
NK = H // 128            # 8 feature chunks
SEG_PER_TILE = 128 // (T // S)  # 16 segments per 128-token tile

ACT = mybir.ActivationFunctionType


def _emit(ctx: ExitStack, tc: "tile.TileContext", out_ap: bass.AP, ins: dict,
          repeat: int = 1):
    """Emit the per-core program. `ins` maps input names -> DRAM APs."""
    nc = tc.nc

    consts = ctx.enter_context(tc.tile_pool(name="consts", bufs=1))
    work = ctx.enter_context(tc.tile_pool(name="work", bufs=1))
    psum = ctx.enter_context(tc.tile_pool(name="psum", bufs=1, space="PSUM"))

    def const_tile(name, shape, dtype=BF16):
        t = consts.tile(shape, dtype, name=name, tag=name)
        nc.sync.dma_start(t[:], ins[name][:])
        return t

    for _rep in range(repeat):
        _emit_once(nc, tc, work, psum, const_tile, out_ap, ins)


def _emit_once(nc, tc, work, psum, const_tile, out_ap, ins):

    # Constants / weights (resident for the whole kernel)
    # ptm[i, p, t, j]: j<16 -> token->segment pooling (1/8); j=16 -> var counts/8
    ptm = const_tile("ptm", [128, IPC * NT * 20])
    w1a, w1b, w2, w3, b1h, b2t, b3 = {}, {}, {}, {}, {}, {}, {}
    for br in ("back", "fwd"):
        w1a[br] = const_tile(f"w1a_{br}", [128, NK * H])    # [p, kc*1024 + fc*128 + f]
        w1b[br] = const_tile(f"w1b_{br}", [128, NK * H])
        w2[br] = const_tile(f"w2_{br}", [128, NK * H])
        w3[br] = const_tile(f"w3_{br}", [128, NK])          # [p, kc]
        b1h[br] = const_tile(f"b1h_{br}", [128, NK], F32)   # [p, fc]
        b2t[br] = const_tile(f"b2t_{br}", [128, NK], F32)   # [p, fc]
        b3[br] = const_tile(f"b3_{br}", [1, 1], F32)

    x_dram = ins["x"]  # [IPC, 2, 128, 8192] bf16, host-pretiled

    logits = []
    for q in range(NQUAD):
        # ---------------- pooling + var means for the quad ----------------
        stmt_t = work.tile([128, 4 * NK * S], BF16, name="stmt_t", tag="stmt_t", bufs=2)
        var32 = work.tile([128, NK * 4], F32, name="var32", tag="var32", bufs=2)
        for iq in range(4):
            item = q * 4 + iq
            for g in range(2):  # two 512-feature column groups
                xbuf = work.tile([128, NT * 512], BF16, name="xbuf", tag="xbuf", bufs=2)
                # SWDGE (gpsimd): HWDGE direct2d DMAs only support 2 sync
                # waits and this slot-reused load needs 3.
                nc.gpsimd.dma_start(xbuf[:], x_dram[item, g])
                for c4 in range(4):
                    c = 4 * g + c4  # global feature chunk 0..7
                    pp = psum.tile([128, NT * 20], F32, name="pp", tag="pool_ps",
                                   bufs=4)
                    for t in range(NT):
                        nc.tensor.matmul(
                            pp[:, 20 * t :][:, :20],
                            xbuf[:, t * 512 + c4 * 128 :][:, :128],
                            ptm[:, (item * NT + t) * 20 :][:, :20],
                            start=True, stop=True,
                        )
                    # stmt means: cols {17t+j, j<16} -> stmtT[:, c*256 + 16t + j]
                    nc.vector.tensor_copy(
                        stmt_t[:, iq * (NK * S) + c * S :][:, :S]
                        .rearrange("p (t j) -> p t j", j=16),
                        pp[:].rearrange("p (t j) -> p t j", j=20)[:, :, 0:16],
                    )
                    # var partials: cols {17t+16} -> reduce over t
                    nc.vector.tensor_reduce(
                        var32[:, c * 4 + iq :][:, :1],
                        pp[:].rearrange("p (t j) -> p t j", j=20)[:, :, 16:17],
                        axis=mybir.AxisListType.XY,
                        op=mybir.AluOpType.add,
                    )
        varb = work.tile([128, NK * 4], BF16, name="varb", tag="varb", bufs=2)
        nc.vector.tensor_copy(varb[:], var32[:])

        # ---------------- MLP for the quad (branches sequential) ----------------
        stmt_r = stmt_t[:].rearrange("p (i r) -> p i r", r=NK * S)
        for br, seg_off in (("back", 0), ("fwd", VAR_LINE)):
            # bias1T[f, item] = (var @ W1b)[item, f] + b1[f]
            b1t = work.tile([128, NK * 4], F32, name="b1t", tag="b1t", bufs=2)
            for fc in range(NK):
                bp = psum.tile([128, 4], F32, name="bp", tag="b1t_ps", bufs=1)
                for kc in range(NK):
                    nc.tensor.matmul(
                        bp[:],
                        w1b[br][:, kc * H + fc * 128 :][:, :128],
                        varb[:, kc * 4 :][:, :4],
                        start=(kc == 0), stop=(kc == NK - 1),
                    )
                nc.vector.tensor_scalar_add(
                    b1t[:, fc * 4 :][:, :4], bp[:], b1h[br][:, fc :][:, :1]
                )
            # layer 1: h1T[f, (item, s)] = gelu(W1a.T @ stmtT + bias1T)
            h1 = work.tile([128, NK * 512], BF16, name="h1", tag="h1", bufs=1)
            for fc in range(NK):
                mp = psum.tile([128, 512], F32, name="mp", tag="mm_ps", bufs=2)
                for kc in range(NK):
                    nc.tensor.matmul(
                        mp[:],
                        w1a[br][:, kc * H + fc * 128 :][:, :128],
                        stmt_r[:, :, kc * S + seg_off :][:, :, :128],
                        start=(kc == 0), stop=(kc == NK - 1),
                    )
                for iq in range(4):
                    nc.scalar.activation(
                        h1[:, fc * 512 + iq * 128 :][:, :128],
                        mp[:, iq * 128 :][:, :128],
                        ACT.Gelu,
                        bias=b1t[:, fc * 4 + iq :][:, :1],
                    )

            # layer 2: h2T = gelu(W2.T @ h1T + b2)
            h2 = work.tile([128, NK * 512], BF16, name="h2", tag="h2", bufs=1)
            for fc in range(NK):
                mp = psum.tile([128, 512], F32, name="mp", tag="mm_ps", bufs=2)
                for kc in range(NK):
                    nc.tensor.matmul(
                        mp[:],
                        w2[br][:, kc * H + fc * 128 :][:, :128],
                        h1[:, kc * 512 :][:, :512],
                        start=(kc == 0), stop=(kc == NK - 1),
                    )
                nc.scalar.activation(
                    h2[:, fc * 512 :][:, :512], mp[:], ACT.Gelu,
                    bias=b2t[br][:, fc :][:, :1],
                )

            # layer 3 logits (sigmoid deferred to a single pass at the end)
            lp = psum.tile([1, 512], F32, name="lp", tag="l3_ps", bufs=1)
            for kc in range(NK):
                nc.tensor.matmul(
                    lp[:], w3[br][:, kc : kc + 1], h2[:, kc * 512 :][:, :512],
                    start=(kc == 0), stop=(kc == NK - 1),
                )
            lg = work.tile([1, 512], F32, name="lg", tag=f"lg{q}_{br}", bufs=1)
            nc.vector.tensor_copy(lg[:], lp[:])
            logits.append((q, br, lg))

    # ---------------- sigmoid + output ----------------
    for q, br, lg in logits:
        preds = work.tile([1, 512], F32, name="preds", tag="preds", bufs=2)
        nc.scalar.activation(preds[:], lg[:], ACT.Sigmoid, bias=b3[br][:, :1])
        for iq in range(4):
            if br == "back":
                nc.sync.dma_start(
                    out_ap[q * 4 + iq, 0:128], preds[:, iq * 128 :][:, :128]
                )
            else:
                nc.sync.dma_start(
                    out_ap[q * 4 + iq, 128 : S - 1],
                    preds[:, iq * 128 + 1 :][:, : S - VAR_LINE - 1],
                )


# ------------------------- host-side preparation -------------------------

def _chunked_w(w):
    """[1024, 1024] -> SBUF lhsT chunk layout [128, kc*1024 + fc*128 + f]."""
    return np.ascontiguousarray(
        w.reshape(8, 128, 8, 128).transpose(1, 0, 2, 3).reshape(128, 8192)
    )


def _prep_weights(inputs):
    g = {}
    for br in ("back", "fwd"):
        w1 = np.asarray(inputs[f"{br}_w1"], np.float32)
        w2 = np.asarray(inputs[f"{br}_w2"], np.float32)
        w3 = np.asarray(inputs[f"{br}_w3"], np.float32)
        g[f"w1a_{br}"] = _chunked_w(w1[:H]).astype(NPBF16)
        g[f"w1b_{br}"] = _chunked_w(w1[H:]).astype(NPBF16)
        g[f"w2_{br}"] = _chunked_w(w2).astype(NPBF16)
        g[f"w3_{br}"] = np.ascontiguousarray(w3.reshape(8, 128).T).astype(NPBF16)
        g[f"b1h_{br}"] = np.ascontiguousarray(
            np.asarray(inputs[f"{br}_b1"], np.float32).reshape(8, 128).T
        )
        g[f"b2t_{br}"] = np.ascontiguousarray(
            np.asarray(inputs[f"{br}_b2"], np.float32).reshape(8, 128).T
        )
        g[f"b3_{br}"] = np.asarray(inputs[f"{br}_b3"], np.float32).reshape(1, 1)
    return g


_PT = np.equal.outer(np.arange(128) // 8, np.arange(SEG_PER_TILE)).astype(np.float32) / 8.0


def _make_ptm(vids_core):
    """[128, IPC, NT, 17]: cols 0..15 token->segment pooling, col 16 var counts/8."""
    ptm = np.zeros((128, IPC, NT, 20), np.float32)
    ptm[:, :, :, :16] = _PT[:, None, None, :]
    for i in range(IPC):
        for v in vids_core[i]:
            t, p = divmod(int(v), 128)
            ptm[p, i, t, 16] += 1.0 / V
    return ptm.reshape(128, IPC * NT * 20).astype(NPBF16)


_CACHE: dict = {}


def _build_program(repeat: int = 1):
    nc = bacc.Bacc("TRN2", target_bir_lowering=False, debug=False)
    shapes = {
        "x": ([IPC, 2, 128, NT * 512], BF16),
        "ptm": ([128, IPC * NT * 20], BF16),
    }
    for br in ("back", "fwd"):
        shapes[f"w1a_{br}"] = ([128, NK * H], BF16)
        shapes[f"w1b_{br}"] = ([128, NK * H], BF16)
        shapes[f"w2_{br}"] = ([128, NK * H], BF16)
        shapes[f"w3_{br}"] = ([128, NK], BF16)
        shapes[f"b1h_{br}"] = ([128, NK], F32)
        shapes[f"b2t_{br}"] = ([128, NK], F32)
        shapes[f"b3_{br}"] = ([1, 1], F32)
    aps = {
        name: nc.dram_tensor(name, shape, dt, kind="ExternalInput").ap()
        for name, (shape, dt) in shapes.items()
    }
    out = nc.dram_tensor("out", [IPC, S - 1], F32, kind="ExternalOutput").ap()
    with tile.TileContext(nc) as tc:
        with ExitStack() as ctx:
            _emit(ctx, tc, out, aps, repeat=repeat)
    nc.compile()
    return nc


def _make_in_maps(inputs):
    x = np.asarray(inputs["hidden_states"], np.float32)
    vids = np.asarray(inputs["variables_ids"], np.int64)
    sids = np.asarray(inputs["statements_ids"], np.int64)
    assert int(inputs["var_line"]) == VAR_LINE and int(inputs["num_statements"]) == S
    expect = np.tile(np.arange(T, dtype=np.int64) // (T // S), (B, 1))
    assert np.array_equal(sids, expect), "statements_ids must be contiguous blocks"

    # Pre-tile for DMA: x_pre[b, g, p, t*512 + c] = x[b, 128*t + p, 512*g + c]
    # so each SBUF partition's load is one contiguous 16 KB strip.
    xb = np.ascontiguousarray(
        x.astype(NPBF16)
        .reshape(B, NT, 128, 2, 512)
        .transpose(0, 3, 2, 1, 4)
        .reshape(B, 2, 128, NT * 512)
    )
    weights = _prep_weights(inputs)

    in_maps = []
    for c in range(NCORES):
        im = dict(weights)
        im["x"] = np.ascontiguousarray(xb[c * IPC : (c + 1) * IPC])
        im["ptm"] = _make_ptm(vids[c * IPC : (c + 1) * IPC])
        in_maps.append(im)
    return in_maps


def _get_nc(repeat=1):
    key = ("nc", repeat)
    if key not in _CACHE:
        _CACHE[key] = _build_program(repeat=repeat)
    return _CACHE[key]


def _run(inputs, trace=False, **kw):
    nc = _get_nc()
    in_maps = _make_in_maps(inputs)
    res = run_bass_kernel_spmd(nc, in_maps, list(range(NCORES)), trace=trace, **kw)
    out = np.concatenate([r["out"] for r in res.results], axis=0).astype(np.float32)
    return out, res


def make_executor(inputs, repeat=1):
    """Build the 8-core shard_map jit once and keep inputs device-resident,
    so repeated calls time dispatch + kernel execution only."""
    import jax
    from jax.sharding import Mesh, PartitionSpec
    from jax.experimental.shard_map import shard_map
    from concourse import bass2jax

    bass2jax.install_neuronx_cc_hook()
    nc = _get_nc(repeat=repeat)
    in_maps = _make_in_maps(inputs)

    import concourse.mybir as mybir_

    partition_name = nc.partition_id_tensor.name if nc.partition_id_tensor else None
    in_names, out_names, out_avals, zero_outs = [], [], [], []
    for alloc in nc.m.functions[0].allocations:
        if not isinstance(alloc, mybir_.MemoryLocationSet):
            continue
        name = alloc.memorylocations[0].name
        if alloc.kind == "ExternalInput":
            if name != partition_name:
                in_names.append(name)
        elif alloc.kind == "ExternalOutput":
            out_names.append(name)
            shape = tuple(alloc.tensor_shape)
            dtype = mybir_.dt.np(alloc.dtype)
            out_avals.append(jax.core.ShapedArray(shape, dtype))
            zero_outs.append(np.zeros(shape, dtype))
    n_params = len(in_names)
    n_outs = len(out_avals)
    all_names = in_names + out_names
    if partition_name is not None:
        all_names = all_names + [partition_name]

    def _body(*args):
        operands = list(args)
        if partition_name is not None:
            operands.append(bass2jax.partition_id_tensor())
        outs = bass2jax._bass_exec_p.bind(
            *operands,
            out_avals=tuple(out_avals),
            in_names=tuple(all_names),
            out_names=tuple(out_names),
            lowering_input_output_aliases=(),
            sim_require_finite=True,
            sim_require_nnan=True,
            nc=nc,
        )
        return tuple(outs)

    devices = jax.devices()[:NCORES]
    mesh = Mesh(np.asarray(devices), ("core",))
    sharded = jax.jit(
        shard_map(
            _body, mesh=mesh,
            in_specs=(PartitionSpec("core"),) * (n_params + n_outs),
            out_specs=(PartitionSpec("core"),) * n_outs,
            check_rep=False,
        ),
        donate_argnums=tuple(range(n_params, n_params + n_outs)),
        keep_unused=True,
    )
    from jax.sharding import NamedSharding

    sh = NamedSharding(mesh, PartitionSpec("core"))
    concat_in = [
        jax.device_put(
            np.concatenate([np.asarray(in_maps[c][nm]) for c in range(NCORES)], axis=0),
            sh,
        )
        for nm in in_names
    ]

    def run():
        zeros = [np.zeros((NCORES * z.shape[0], *z.shape[1:]), z.dtype) for z in zero_outs]
        out_arrs = sharded(*concat_in, *zeros)
        jax.block_until_ready(out_arrs)
        return np.asarray(out_arrs[0]).reshape(NCORES, IPC, S - 1).reshape(B, S - 1)

    return run


def kernel(**inputs) -> np.ndarray:
    out, _ = _run(inputs)
    return out
